# revision 1
# baseline (speedup 1.0000x reference)
"""Trainium2 Bass kernel for nn_Graph_to_Featuremaps_savemem.

Reference computation:
    scores[b,p,n] = s_res[b,p] + s_hid[b,n];  attn = softmax_n(scores)
    out[b,c,p]    = relu(sum_n attn[b,p,n] * (x[b,n,:] @ W)[c])

Key simplification: softmax over n is shift-invariant, so the per-pixel
s_res[b,p] term (the only use of res_feature / node_fea_for_res) cancels:
    attn[b,p,n] = softmax_n(s_hid[b,n])   (independent of p)
    out[b,c,p]  = relu(sum_n a[b,n] * nv[b,n,c])  broadcast over all pixels.

So the kernel is a tiny softmax-weighted matmul (per-batch (7,256)x(256,256))
followed by a 151 MB broadcast-write of the (B,C) result over H*W pixels.
Sharding: data-parallel over batch, 2 batches per core across 8 cores; the
small params (node_fea_for_hidden, weight) are replicated.

Hardware constraints shaping the structure:
- PE matmul / tensor-scalar / DMA-trigger instructions have a single
  sync-wait slot, so every PE operand pair must share one producer
  semaphore. All small inputs (w, x, nfh, identity, block-mask, ones) are
  packed host-side into ONE DRAM tensor loaded by ONE DMA; PSUM results are
  funneled through DVE copies.
- The kernel-tail drain also has limited wait slots, so the kernel keeps the
  total semaphore count low: only ACT (which triggers all DMAs), PE, DVE and
  the 8 HW DMA queues are used.
- matmul operands need base partition 0/32/64; x and the transpose identity
  live at rows 32:46 of the packed tile, everything else at base 0.
"""

import numpy as np

import concourse.bass as bass
import concourse.mybir as mybir
import concourse.tile as tile
from concourse.bass_utils import run_bass_kernel_spmd

B, NODES, HID, C, H, W = 16, 7, 256, 256, 96, 96
P = H * W                # 9216 pixels
NCORES = 8
BL = B // NCORES         # 2 local batches per core
BN = BL * NODES          # 14 (b,n) rows
WCHUNK = 9216            # broadcast tile width; P = 1 * WCHUNK
NCHUNK = P // WCHUNK

# Packed input layout: (128, CIN_COLS) float32
COL_W = 0        # cols 0:512, all rows: w[kh*128+k, c] at [k, kh*256+c]
COL_ID = 512     # cols 512:526, rows 32:46: identity(14)
COL_BM = 526     # cols 526:528, rows 0:14: block-diagonal mask (14, 2)
COL_XN = 528     # cols 528:784: row 0 = nfh; rows 32:46 = x[(b n), h]
COL_ONE = 784    # col 784, row 0: 1.0
CIN_COLS = 785
XROW = 32        # base partition for x / identity (must be 0, 32 or 64)

_cache: dict = {}


def _build_nc():
    nc = bass.Bass()
    dt = mybir.dt.float32
    cin_d = nc.declare_dram_parameter("cin", [128, CIN_COLS], dt, isOutput=False)
    out_d = nc.declare_dram_parameter("out", [BL, C, P], dt, isOutput=True)

    with tile.TileContext(nc) as tc:
        with (
            tc.tile_pool(name="sb", bufs=1) as sb,
            tc.tile_pool(name="ps", bufs=1, space=bass.MemorySpace.PSUM) as ps,
        ):
            cin = sb.tile([128, CIN_COLS], dt)
            nc.scalar.dma_start(out=cin[:], in_=cin_d[:])
            x_sl = cin[XROW : XROW + BN, COL_XN : COL_XN + HID]
            ident = cin[XROW : XROW + BN, COL_ID : COL_ID + BN]
            nfh_row = cin[0:1, COL_XN : COL_XN + HID]
            one_cin = cin[0:1, COL_ONE : COL_ONE + 1]

            ones11 = sb.tile([1, 1], dt)
            nc.vector.memset(ones11[:], 1.0)
            sb_w = sb.tile([128, 2 * C], dt)
            nc.vector.tensor_copy(out=sb_w[:], in_=cin[:, 0 : 2 * C])
            blkmask = sb.tile([BN, BL], dt)
            nc.vector.tensor_copy(out=blkmask[:], in_=cin[0:BN, COL_BM : COL_BM + BL])

            # PE-transpose x to (h, bn) layout, one (128, 14) tile per k-half.
            sbT = []
            for kh in range(2):
                p_t = ps.tile([128, BN], dt, tag=f"xT{kh}")
                nc.tensor.transpose(p_t[:], x_sl[:, kh * 128 : (kh + 1) * 128], ident)
                s_t = sb.tile([128, BN], dt, tag=f"sbT{kh}")
                nc.vector.tensor_copy(out=s_t[:], in_=p_t[:])
                sbT.append(s_t)
            # Transpose nfh row to a (128, kh) column pair via K=1 matmuls.
            p_nfh = ps.tile([128, 2], dt, tag="nfhT")
            for kh in range(2):
                nc.tensor.matmul(
                    p_nfh[:, kh : kh + 1],
                    nfh_row[:, kh * 128 : (kh + 1) * 128],
                    one_cin,
                    start=True,
                    stop=True,
                )
            sb_nfh_col = sb.tile([128, 2], dt)
            nc.vector.tensor_copy(out=sb_nfh_col[:], in_=p_nfh[:])

            # s_hid row (1, 14) and node_vals (14, 256), contracting h in 2 halves.
            ps_s = ps.tile([1, BN], dt, tag="s")
            ps_nv = ps.tile([BN, C], dt, tag="nv")
            for kh in range(2):
                nc.tensor.matmul(
                    ps_s[:],
                    sb_nfh_col[:, kh : kh + 1],
                    sbT[kh][:],
                    start=(kh == 0),
                    stop=(kh == 1),
                )
                nc.tensor.matmul(
                    ps_nv[:],
                    sbT[kh][:],
                    sb_w[:, kh * C : (kh + 1) * C],
                    start=(kh == 0),
                    stop=(kh == 1),
                )
            sb_nv = sb.tile([BN, C], dt)
            nc.vector.tensor_copy(out=sb_nv[:], in_=ps_nv[:])

            # Softmax over the 7 nodes (free dim), separately per local batch.
            e_row = sb.tile([1, BN], dt)
            denom = sb.tile([1, BL], dt)
            recip = sb.tile([1, BL], dt)
            a_row = sb.tile([1, BN], dt)
            for b in range(BL):
                nc.scalar.activation(
                    e_row[:, b * NODES : (b + 1) * NODES],
                    ps_s[:, b * NODES : (b + 1) * NODES],
                    mybir.ActivationFunctionType.Exp,
                    accum_out=denom[:, b : b + 1],
                )
            nc.vector.reciprocal(recip[:], denom[:])
            for b in range(BL):
                nc.vector.tensor_scalar_mul(
                    a_row[:, b * NODES : (b + 1) * NODES],
                    e_row[:, b * NODES : (b + 1) * NODES],
                    recip[:, b : b + 1],
                )

            # Transpose attn row to a column via K=1 matmul: ps_a[(b,n), 0] = a[b, n],
            # then expand into a block-diagonal (14, BL) matrix so one matmul per
            # c-half computes v for both local batches.
            ps_a = ps.tile([BN, 1], dt, tag="a")
            nc.tensor.matmul(ps_a[:], a_row[:], ones11[:], start=True, stop=True)
            sb_a = sb.tile([BN, 1], dt)
            nc.vector.tensor_copy(out=sb_a[:], in_=ps_a[:])
            rhs_a = sb.tile([BN, BL], dt)
            nc.vector.tensor_scalar_mul(rhs_a[:], blkmask[:], sb_a[:])

            # v[c, (ch, b)] = sum_n a[b, n] * nv[(b,n), c]; relu; broadcast; store.
            ps_v = ps.tile([128, 2 * BL], dt, tag="v")
            for ch in range(2):
                nc.tensor.matmul(
                    ps_v[:, ch * BL : (ch + 1) * BL],
                    sb_nv[:, ch * 128 : (ch + 1) * 128],
                    rhs_a[:],
                    start=True,
                    stop=True,
                )
            sb_v = sb.tile([128, 2 * BL], dt)
            nc.scalar.activation(sb_v[:], ps_v[:], mybir.ActivationFunctionType.Relu)
            # One broadcast tile + one DMA per local batch: out[b] is (256, P)
            # contiguous in DRAM, viewed as [p, ch, pix] with c = ch*128 + p.
            # Two DMAs let batch 1's broadcast fills overlap batch 0's store;
            # _fix_tail_drain spreads the resulting queue waits over spare
            # zero-wait tail drains.
            for b in range(BL):
                bc = sb.tile([128, 2, P], dt, tag=f"bc{b}")
                for ch in range(2):
                    j = ch * BL + b
                    nc.vector.tensor_copy(
                        out=bc[:, ch, :], in_=sb_v[:, j : j + 1].to_broadcast([128, P])
                    )
                nc.scalar.dma_start(
                    out=out_d[b].rearrange("(ch p) pix -> p ch pix", p=128),
                    in_=bc[:],
                )
    _fix_tail_drain(nc)
    return nc


def _fix_tail_drain(nc):
    """Walrus in this toolchain accepts very few sync waits per instruction, and
    Tile's kernel-tail drain waits on every semaphore. In this kernel the whole
    dataflow is one chain ending in the single output DMA: every other sem tick
    (input-DMA queue, PE, DVE, ACT) is strictly upstream of the output-DMA
    trigger, so waiting on the output queue's completion sem alone is
    sufficient. Strip the drain down to that one wait."""
    import bass_rust

    out_sem = None
    for ins in nc.inst_map.values():
        if type(ins).__name__ == "InstDMACopy" and "out_set" in str(ins):
            si = ins.sync_info
            if si is not None and len(si.on_update) > 0:
                out_sem = si.on_update[0].ant_name
    assert out_sem is not None, "output DMA completion sem not found"
    for ins in nc.inst_map.values():
        si = ins.sync_info
        if type(ins).__name__ == "InstDrain" and si is not None and len(si.on_wait) > 1:
            keep = [w for w in si.on_wait if w.ant_name == out_sem]
            assert len(keep) == 1, (out_sem, [w.ant_name for w in si.on_wait])
            ins.sync_info = bass_rust.SyncInfo(
                on_wait=keep, on_update=list(si.on_update)
            )


def _get_nc():
    if "nc" not in _cache:
        _cache["nc"] = _build_nc()
    return _cache["nc"]


def _pack_cin(x_shard, nfh, w):
    """Pack one core's inputs into the (128, CIN_COLS) tensor."""
    cin = np.zeros((128, CIN_COLS), dtype=np.float32)
    # w: [kh*128+k, c] -> [k, kh*256+c]
    cin[:, 0:C] = w[0:128, :]
    cin[:, C : 2 * C] = w[128:256, :]
    cin[XROW : XROW + BN, COL_ID : COL_ID + BN] = np.eye(BN, dtype=np.float32)
    for b in range(BL):
        cin[b * NODES : (b + 1) * NODES, COL_BM + b] = 1.0
    cin[0, COL_XN : COL_XN + HID] = nfh[:, 0]
    cin[XROW : XROW + BN, COL_XN : COL_XN + HID] = x_shard.reshape(BN, HID)
    cin[0, COL_ONE] = 1.0
    return cin


def _make_in_maps(input, node_fea_for_hidden, weight):
    x_full = np.asarray(input, dtype=np.float32)[0]  # (B, N, HID)
    nfh = np.asarray(node_fea_for_hidden, dtype=np.float32)
    w = np.asarray(weight, dtype=np.float32)
    return [
        {"cin": _pack_cin(x_full[i * BL : (i + 1) * BL], nfh, w)}
        for i in range(NCORES)
    ]


def _run(in_maps, trace=False, **kwargs):
    nc = _get_nc()
    return run_bass_kernel_spmd(nc, in_maps, list(range(NCORES)), trace=trace, **kwargs)


def kernel(input, res_feature, node_fea_for_res, node_fea_for_hidden, weight):
    in_maps = _make_in_maps(input, node_fea_for_hidden, weight)
    res = _run(in_maps)
    shards = [res.results[i]["out"] for i in range(NCORES)]  # each (BL, C, P)
    full = np.concatenate(shards, axis=0)  # (B, C, P)
    return full.reshape(B, C, H, W).astype(np.float32, copy=False)



# revision 7
# speedup vs baseline: 1.0110x; 1.0110x over previous
"""Trainium2 Bass kernel for nn_Graph_to_Featuremaps_savemem.

Reference computation:
    scores[b,p,n] = s_res[b,p] + s_hid[b,n];  attn = softmax_n(scores)
    out[b,c,p]    = relu(sum_n attn[b,p,n] * (x[b,n,:] @ W)[c])

Key simplification: softmax over n is shift-invariant, so the per-pixel
s_res[b,p] term (the only use of res_feature / node_fea_for_res) cancels:
    attn[b,p,n] = softmax_n(s_hid[b,n])   (independent of p)
    out[b,c,p]  = relu(sum_n a[b,n] * nv[b,n,c])  broadcast over all pixels.

So the kernel is a tiny softmax-weighted matmul (per-batch (7,256)x(256,256))
followed by an 18.9 MB-per-core broadcast-write of the (BL,C) result over
H*W pixels.  Sharding: data-parallel over batch, 2 batches per core across
8 cores; the small params (node_fea_for_hidden, weight) are replicated.

The store is the roofline: ~44.5 us of DMA at the ~424 GB/s per-core cap.
Everything else is head/tail latency, minimized as follows:
- x is packed host-side already transposed (h-major), so no PE transposes
  or PSUM round-trips before the first matmul; all matmul operands come
  straight out of the single packed input tile.
- The input DMA is triggered from the Vector engine, whose preamble
  finishes earliest, so input bytes land ~2 us sooner than a Scalar
  trigger would allow.
- Softmax is computed in column layout ((b,n) on partitions): exp on ACT,
  per-batch denominators via a block-diagonal matmul, reciprocal+scale on
  DVE.  The attn column never needs transposing.
- The broadcast fill (DVE, ~950 GB/s) and the store DMA (~424 GB/s) are
  pipelined in chunks along the pixel axis, with small first chunks so the
  first store triggers right after a ~1.2 MB fill instead of after a full
  9.4 MB batch fill.

Hardware constraints shaping the structure:
- PE matmul / tensor-scalar / DMA-trigger instructions have a single
  sync-wait slot, so every PE operand pair must share one producer
  semaphore: cin+cin (input queue sem), e_col+blk2 (both ACT: blk2 is
  copied out of cin by ACT), sb_nv+ablk (both DVE).
- The kernel-tail drain also has limited wait slots; _fix_tail_drain
  strips it to the one semaphore whose completion implies all others
  (the last store's queue sem).
"""

import numpy as np

import concourse.bass as bass
import concourse.mybir as mybir
import concourse.tile as tile
from concourse.bass_utils import run_bass_kernel_spmd

B, NODES, HID, C, H, W = 16, 7, 256, 256, 96, 96
P = H * W                # 9216 pixels
NCORES = 8
BL = B // NCORES         # 2 local batches per core
BN = BL * NODES          # 14 (b,n) rows

# Pixel-axis chunking of the fill->store pipeline.  First chunks are small
# so the first DMA triggers early; later chunks are bigger because Tile has
# only 8 DMAHW semaphores and reusing one puts a second sync wait on the
# trigger (walrus allows one): 1 input DMA + 7 stores = 8 total.
CHUNKS = {
    0: [576, 1152, 3456, 4032],
    1: [3072, 3072, 3072],
}
assert all(sum(v) == P for v in CHUNKS.values())

# Packed input layout: (128, CIN_COLS) float32
COL_W = 0         # cols 0:512, all rows: w[kh*128+k, c] at [k, kh*256+c]
COL_XT = 512      # cols 512:540: xT[k, kh*BN + (b n)] = x[(b n), kh*128+k]
COL_NFH = 540     # cols 540:542: nfh[kh*128+k] at [k, kh]
COL_BLK2 = 542    # cols 542:556, rows 0:14: block-diag ones(7,7) x2
COL_BM = 556      # cols 556:558, rows 0:14: block mask [(b n), b]
CIN_COLS = 558

_cache: dict = {}


def _build_nc():
    nc = bass.Bass()
    dt = mybir.dt.float32
    cin_d = nc.declare_dram_parameter("cin", [128, CIN_COLS], dt, isOutput=False)
    out_d = nc.declare_dram_parameter("out", [BL, C, P], dt, isOutput=True)

    with tile.TileContext(nc) as tc:
        with (
            tc.tile_pool(name="sb", bufs=1) as sb,
            tc.tile_pool(name="ps", bufs=1, space=bass.MemorySpace.PSUM) as ps,
        ):
            cin = sb.tile([128, CIN_COLS], dt)
            # Sync (SP) is a hardware-DGE trigger engine and is otherwise
            # idle, so the input DMA does not queue behind Scalar's preamble.
            nc.sync.dma_start(out=cin[:], in_=cin_d[:])

            xt = [cin[:, COL_XT + kh * BN : COL_XT + (kh + 1) * BN] for kh in (0, 1)]
            nfh = [cin[:, COL_NFH + kh : COL_NFH + kh + 1] for kh in (0, 1)]
            wh = [cin[:, kh * C : (kh + 1) * C] for kh in (0, 1)]
            blk2_cin = cin[0:BN, COL_BLK2 : COL_BLK2 + BN]
            blkmask = cin[0:BN, COL_BM : COL_BM + BL]

            # ACT copies blk2 so the denominator matmul's operands (e_col,
            # blk2) share the ACT semaphore.
            sc_blk2 = sb.tile([BN, BN], dt)
            nc.scalar.copy(out=sc_blk2[:], in_=blk2_cin)

            # s_col[(b n), 0] = sum_h x[(b n), h] * nfh[h]; nv = x @ W.
            ps_s = ps.tile([BN, 1], dt, tag="s")
            ps_nv = ps.tile([BN, C], dt, tag="nv")
            for kh in range(2):
                nc.tensor.matmul(
                    ps_s[:], xt[kh], nfh[kh], start=(kh == 0), stop=(kh == 1)
                )
            for kh in range(2):
                nc.tensor.matmul(
                    ps_nv[:], xt[kh], wh[kh], start=(kh == 0), stop=(kh == 1)
                )

            # Softmax over the 7 nodes of each local batch, in column layout.
            e_col = sb.tile([BN, 1], dt)
            nc.scalar.activation(e_col[:], ps_s[:], mybir.ActivationFunctionType.Exp)
            ps_den = ps.tile([BN, 1], dt, tag="den")
            nc.tensor.matmul(ps_den[:], sc_blk2[:], e_col[:], start=True, stop=True)

            # DVE stream.  Walrus accepts a single sync wait per instruction,
            # so each cross-engine semaphore is acquired once by a plain copy
            # (one free wait slot); later DVE instructions are covered
            # transitively.  TensorScalarPtr needs in0 AND the ptr operand
            # DVE-produced so its two waits merge into one DVE-sem wait.
            sb_bm = sb.tile([BN, BL], dt)
            nc.vector.tensor_copy(out=sb_bm[:], in_=blkmask)       # DMAHW wait
            sb_nv = sb.tile([BN, C], dt)
            nc.vector.tensor_copy(out=sb_nv[:], in_=ps_nv[:])      # PE wait
            recip = sb.tile([BN, 1], dt)
            nc.vector.reciprocal(recip[:], ps_den[:])              # PE wait
            sb_e = sb.tile([BN, 1], dt)
            nc.vector.tensor_copy(out=sb_e[:], in_=e_col[:])       # ACT wait
            t_col = sb.tile([BN, 1], dt)
            nc.vector.tensor_scalar_mul(t_col[:], sb_e[:], recip[:])
            # ablk[(b n), b'] = attn[(b n)] if b == b' else 0.
            ablk = sb.tile([BN, BL], dt)
            nc.vector.tensor_scalar_mul(ablk[:], sb_bm[:], t_col[:])

            # v[c, ch*BL + b] = sum_n attn[b, n] * nv[(b n), c], c = ch*128+p.
            ps_v = ps.tile([128, 2 * BL], dt, tag="v")
            for ch in range(2):
                nc.tensor.matmul(
                    ps_v[:, ch * BL : (ch + 1) * BL],
                    sb_nv[:, ch * 128 : (ch + 1) * 128],
                    ablk[:],
                    start=True,
                    stop=True,
                )
            sb_v = sb.tile([128, 2 * BL], dt)
            nc.scalar.activation(sb_v[:], ps_v[:], mybir.ActivationFunctionType.Relu)

            # Pipelined broadcast fill (DVE) -> store (DMA) over pixel chunks.
            # out[b] is (256, P) in DRAM, viewed as [p, ch, pix], c = ch*128+p.
            for b in range(BL):
                o = 0
                for k, chw in enumerate(CHUNKS[b]):
                    bc = sb.tile([128, 2, chw], dt, tag=f"bc{b}_{k}")
                    for ch in range(2):
                        j = ch * BL + b
                        nc.vector.tensor_copy(
                            out=bc[:, ch, :],
                            in_=sb_v[:, j : j + 1].to_broadcast([128, chw]),
                        )
                    nc.scalar.dma_start(
                        out=out_d[b, :, o : o + chw].rearrange(
                            "(ch p) pix -> p ch pix", p=128
                        ),
                        in_=bc[:],
                    )
                    o += chw
    _fix_tail_drain(nc)
    return nc


def _fix_tail_drain(nc):
    """Walrus in this toolchain accepts very few sync waits per instruction,
    and Tile's kernel-tail drain waits on every semaphore.  The dataflow is
    one chain ending in the store DMAs, which all share one hardware queue:
    each of the queue's engines processes its descriptors in FIFO order, so
    the LAST store's completion semaphore reaching its target implies every
    earlier packet (and everything upstream of the triggers) is done.  Strip
    the drain down to that one wait."""
    import bass_rust

    out_sem = None
    for ins in nc.inst_map.values():
        if type(ins).__name__ == "InstDMACopy" and "out_set" in str(ins):
            si = ins.sync_info
            if si is not None and len(si.on_update) > 0:
                out_sem = si.on_update[0].ant_name
    assert out_sem is not None, "output DMA completion sem not found"
    for ins in nc.inst_map.values():
        si = ins.sync_info
        if type(ins).__name__ == "InstDrain" and si is not None and len(si.on_wait) > 1:
            keep = [w for w in si.on_wait if w.ant_name == out_sem]
            assert len(keep) == 1, (out_sem, [w.ant_name for w in si.on_wait])
            ins.sync_info = bass_rust.SyncInfo(
                on_wait=keep, on_update=list(si.on_update)
            )


def _get_nc():
    if "nc" not in _cache:
        _cache["nc"] = _build_nc()
    return _cache["nc"]


def _pack_cin(x_shard, nfh, w):
    """Pack one core's inputs into the (128, CIN_COLS) tensor."""
    cin = np.zeros((128, CIN_COLS), dtype=np.float32)
    x2 = x_shard.reshape(BN, HID)
    for kh in range(2):
        cin[:, kh * C : (kh + 1) * C] = w[kh * 128 : (kh + 1) * 128, :]
        cin[:, COL_XT + kh * BN : COL_XT + (kh + 1) * BN] = x2[
            :, kh * 128 : (kh + 1) * 128
        ].T
        cin[:, COL_NFH + kh] = nfh[kh * 128 : (kh + 1) * 128, 0]
    for b in range(BL):
        r = slice(b * NODES, (b + 1) * NODES)
        cin[r, COL_BLK2 + b * NODES : COL_BLK2 + (b + 1) * NODES] = 1.0
        cin[r, COL_BM + b] = 1.0
    return cin


def _make_in_maps(input, node_fea_for_hidden, weight):
    x_full = np.asarray(input, dtype=np.float32)[0]  # (B, N, HID)
    nfh = np.asarray(node_fea_for_hidden, dtype=np.float32)
    w = np.asarray(weight, dtype=np.float32)
    return [
        {"cin": _pack_cin(x_full[i * BL : (i + 1) * BL], nfh, w)}
        for i in range(NCORES)
    ]


def _run(in_maps, trace=False, **kwargs):
    nc = _get_nc()
    return run_bass_kernel_spmd(nc, in_maps, list(range(NCORES)), trace=trace, **kwargs)


def kernel(input, res_feature, node_fea_for_res, node_fea_for_hidden, weight):
    in_maps = _make_in_maps(input, node_fea_for_hidden, weight)
    res = _run(in_maps)
    shards = [res.results[i]["out"] for i in range(NCORES)]  # each (BL, C, P)
    full = np.concatenate(shards, axis=0)  # (B, C, P)
    return full.reshape(B, C, H, W).astype(np.float32, copy=False)


# revision 8
# speedup vs baseline: 1.1474x; 1.1350x over previous
"""Trainium2 Bass kernel for nn_Graph_to_Featuremaps_savemem.

Reference computation:
    scores[b,p,n] = s_res[b,p] + s_hid[b,n];  attn = softmax_n(scores)
    out[b,c,p]    = relu(sum_n attn[b,p,n] * (x[b,n,:] @ W)[c])

Key simplification: softmax over n is shift-invariant, so the per-pixel
s_res[b,p] term (the only use of res_feature / node_fea_for_res) cancels:
    attn[b,p,n] = softmax_n(s_hid[b,n])   (independent of p)
    out[b,c,p]  = relu(sum_n a[b,n] * nv[b,n,c])  broadcast over all pixels.

So the kernel is a tiny softmax-weighted matmul (per-batch (7,256)x(256,256))
followed by an 18.9 MB-per-core broadcast-write of the (BL,C) result over
H*W pixels.  Sharding: data-parallel over batch, 2 batches per core across
8 cores; the small params (node_fea_for_hidden, weight) are replicated.

The store is the roofline: ~44.5 us of DMA at the ~424 GB/s per-core cap.
Everything else is head/tail latency, minimized as follows:
- x is packed host-side already transposed (h-major), so no PE transposes
  or PSUM round-trips before the first matmul; all matmul operands come
  straight out of the single packed input tile.
- The input DMA is triggered from the Vector engine, whose preamble
  finishes earliest, so input bytes land ~2 us sooner than a Scalar
  trigger would allow.
- Softmax is computed in column layout ((b,n) on partitions): exp on ACT,
  per-batch denominators via a block-diagonal matmul, reciprocal+scale on
  DVE.  The attn column never needs transposing.
- The broadcast fill (DVE, ~950 GB/s) and the store DMA (~424 GB/s) are
  pipelined in chunks along the pixel axis, with small first chunks so the
  first store triggers right after a ~1.2 MB fill instead of after a full
  9.4 MB batch fill.

Hardware constraints shaping the structure:
- PE matmul / tensor-scalar / DMA-trigger instructions have a single
  sync-wait slot, so every PE operand pair must share one producer
  semaphore: cin+cin (input queue sem), e_col+blk2 (both ACT: blk2 is
  copied out of cin by ACT), sb_nv+ablk (both DVE).
- The kernel-tail drain also has limited wait slots; _fix_tail_drain
  strips it to the one semaphore whose completion implies all others
  (the last store's queue sem).
"""

import numpy as np

import concourse.bass as bass
import concourse.mybir as mybir
import concourse.tile as tile
from concourse.bass_utils import run_bass_kernel_spmd

B, NODES, HID, C, H, W = 16, 7, 256, 256, 96, 96
P = H * W                # 9216 pixels
NCORES = 8
BL = B // NCORES         # 2 local batches per core
BN = BL * NODES          # 14 (b,n) rows

# Pixel-axis chunking of the fill->store pipeline.  First chunks are small
# so the first DMA triggers early; later chunks are bigger because Tile has
# only 8 DMAHW semaphores and reusing one puts a second sync wait on the
# trigger (walrus allows one): 1 input DMA + 7 stores = 8 total.
CHUNKS = {
    0: [576, 1152, 3456, 4032],
    1: [3072, 3072, 3072],
}
assert all(sum(v) == P for v in CHUNKS.values())

# Packed input layout: (128, CIN_COLS) float32
COL_W = 0         # cols 0:512, all rows: w[kh*128+k, c] at [k, kh*256+c]
COL_XT = 512      # cols 512:540: xT[k, kh*BN + (b n)] = x[(b n), kh*128+k]
COL_NFH = 540     # cols 540:542: nfh[kh*128+k] at [k, kh]
COL_BLK2 = 542    # cols 542:556, rows 0:14: block-diag ones(7,7) x2
COL_BM = 556      # cols 556:558, rows 0:14: block mask [(b n), b]
CIN_COLS = 558

_cache: dict = {}


def _build_nc():
    nc = bass.Bass()
    dt = mybir.dt.float32
    cin_d = nc.declare_dram_parameter("cin", [128, CIN_COLS], dt, isOutput=False)
    out_d = nc.declare_dram_parameter("out", [BL, C, P], dt, isOutput=True)

    with tile.TileContext(nc) as tc:
        with (
            tc.tile_pool(name="sb", bufs=1) as sb,
            tc.tile_pool(name="ps", bufs=1, space=bass.MemorySpace.PSUM) as ps,
        ):
            cin = sb.tile([128, CIN_COLS], dt)
            nc.scalar.dma_start(out=cin[:], in_=cin_d[:])

            xt = [cin[:, COL_XT + kh * BN : COL_XT + (kh + 1) * BN] for kh in (0, 1)]
            nfh = [cin[:, COL_NFH + kh : COL_NFH + kh + 1] for kh in (0, 1)]
            wh = [cin[:, kh * C : (kh + 1) * C] for kh in (0, 1)]
            blk2_cin = cin[0:BN, COL_BLK2 : COL_BLK2 + BN]
            blkmask = cin[0:BN, COL_BM : COL_BM + BL]

            # ACT copies blk2 so the denominator matmul's operands (e_col,
            # blk2) share the ACT semaphore.
            sc_blk2 = sb.tile([BN, BN], dt)
            nc.scalar.copy(out=sc_blk2[:], in_=blk2_cin)

            # s_col[(b n), 0] = sum_h x[(b n), h] * nfh[h]; nv = x @ W.
            ps_s = ps.tile([BN, 1], dt, tag="s")
            ps_nv = ps.tile([BN, C], dt, tag="nv")
            for kh in range(2):
                nc.tensor.matmul(
                    ps_s[:], xt[kh], nfh[kh], start=(kh == 0), stop=(kh == 1)
                )
            for kh in range(2):
                nc.tensor.matmul(
                    ps_nv[:], xt[kh], wh[kh], start=(kh == 0), stop=(kh == 1)
                )

            # Softmax over the 7 nodes of each local batch, in column layout.
            e_col = sb.tile([BN, 1], dt)
            nc.scalar.activation(e_col[:], ps_s[:], mybir.ActivationFunctionType.Exp)
            ps_den = ps.tile([BN, 1], dt, tag="den")
            nc.tensor.matmul(ps_den[:], sc_blk2[:], e_col[:], start=True, stop=True)

            # DVE stream.  Walrus accepts a single sync wait per instruction,
            # so each cross-engine semaphore is acquired once by a plain copy
            # (one free wait slot); later DVE instructions are covered
            # transitively.  TensorScalarPtr needs in0 AND the ptr operand
            # DVE-produced so its two waits merge into one DVE-sem wait.
            sb_bm = sb.tile([BN, BL], dt)
            nc.vector.tensor_copy(out=sb_bm[:], in_=blkmask)       # DMAHW wait
            sb_nv = sb.tile([BN, C], dt)
            nc.vector.tensor_copy(out=sb_nv[:], in_=ps_nv[:])      # PE wait
            recip = sb.tile([BN, 1], dt)
            nc.vector.reciprocal(recip[:], ps_den[:])              # PE wait
            sb_e = sb.tile([BN, 1], dt)
            nc.vector.tensor_copy(out=sb_e[:], in_=e_col[:])       # ACT wait
            t_col = sb.tile([BN, 1], dt)
            nc.vector.tensor_scalar_mul(t_col[:], sb_e[:], recip[:])
            # ablk[(b n), b'] = attn[(b n)] if b == b' else 0.
            ablk = sb.tile([BN, BL], dt)
            nc.vector.tensor_scalar_mul(ablk[:], sb_bm[:], t_col[:])

            # v[c, ch*BL + b] = sum_n attn[b, n] * nv[(b n), c], c = ch*128+p.
            ps_v = ps.tile([128, 2 * BL], dt, tag="v")
            for ch in range(2):
                nc.tensor.matmul(
                    ps_v[:, ch * BL : (ch + 1) * BL],
                    sb_nv[:, ch * 128 : (ch + 1) * 128],
                    ablk[:],
                    start=True,
                    stop=True,
                )
            sb_v = sb.tile([128, 2 * BL], dt)
            nc.scalar.activation(sb_v[:], ps_v[:], mybir.ActivationFunctionType.Relu)

            # Pipelined broadcast fill (DVE) -> store (DMA) over pixel chunks.
            # out[b] is (256, P) in DRAM, viewed as [p, ch, pix], c = ch*128+p.
            for b in range(BL):
                o = 0
                for k, chw in enumerate(CHUNKS[b]):
                    bc = sb.tile([128, 2, chw], dt, tag=f"bc{b}_{k}")
                    for ch in range(2):
                        j = ch * BL + b
                        nc.vector.tensor_copy(
                            out=bc[:, ch, :],
                            in_=sb_v[:, j : j + 1].to_broadcast([128, chw]),
                        )
                    nc.scalar.dma_start(
                        out=out_d[b, :, o : o + chw].rearrange(
                            "(ch p) pix -> p ch pix", p=128
                        ),
                        in_=bc[:],
                    )
                    o += chw
    _fix_tail_drain(nc)
    return nc


def _fix_tail_drain(nc):
    """Walrus in this toolchain accepts very few sync waits per instruction,
    and Tile's kernel-tail drain waits on every semaphore.  The dataflow is
    one chain ending in the store DMAs, which all share one hardware queue:
    each of the queue's engines processes its descriptors in FIFO order, so
    the LAST store's completion semaphore reaching its target implies every
    earlier packet (and everything upstream of the triggers) is done.  Strip
    the drain down to that one wait."""
    import bass_rust

    out_sem = None
    for ins in nc.inst_map.values():
        if type(ins).__name__ == "InstDMACopy" and "out_set" in str(ins):
            si = ins.sync_info
            if si is not None and len(si.on_update) > 0:
                out_sem = si.on_update[0].ant_name
    assert out_sem is not None, "output DMA completion sem not found"
    for ins in nc.inst_map.values():
        si = ins.sync_info
        if type(ins).__name__ == "InstDrain" and si is not None and len(si.on_wait) > 1:
            keep = [w for w in si.on_wait if w.ant_name == out_sem]
            assert len(keep) == 1, (out_sem, [w.ant_name for w in si.on_wait])
            ins.sync_info = bass_rust.SyncInfo(
                on_wait=keep, on_update=list(si.on_update)
            )


def _get_nc():
    if "nc" not in _cache:
        _cache["nc"] = _build_nc()
    return _cache["nc"]


def _pack_cin(x_shard, nfh, w):
    """Pack one core's inputs into the (128, CIN_COLS) tensor."""
    cin = np.zeros((128, CIN_COLS), dtype=np.float32)
    x2 = x_shard.reshape(BN, HID)
    for kh in range(2):
        cin[:, kh * C : (kh + 1) * C] = w[kh * 128 : (kh + 1) * 128, :]
        cin[:, COL_XT + kh * BN : COL_XT + (kh + 1) * BN] = x2[
            :, kh * 128 : (kh + 1) * 128
        ].T
        cin[:, COL_NFH + kh] = nfh[kh * 128 : (kh + 1) * 128, 0]
    for b in range(BL):
        r = slice(b * NODES, (b + 1) * NODES)
        cin[r, COL_BLK2 + b * NODES : COL_BLK2 + (b + 1) * NODES] = 1.0
        cin[r, COL_BM + b] = 1.0
    return cin


def _make_in_maps(input, node_fea_for_hidden, weight):
    x_full = np.asarray(input, dtype=np.float32)[0]  # (B, N, HID)
    nfh = np.asarray(node_fea_for_hidden, dtype=np.float32)
    w = np.asarray(weight, dtype=np.float32)
    return [
        {"cin": _pack_cin(x_full[i * BL : (i + 1) * BL], nfh, w)}
        for i in range(NCORES)
    ]


def _run(in_maps, trace=False, **kwargs):
    nc = _get_nc()
    return run_bass_kernel_spmd(nc, in_maps, list(range(NCORES)), trace=trace, **kwargs)


def kernel(input, res_feature, node_fea_for_res, node_fea_for_hidden, weight):
    in_maps = _make_in_maps(input, node_fea_for_hidden, weight)
    res = _run(in_maps)
    shards = [res.results[i]["out"] for i in range(NCORES)]  # each (BL, C, P)
    full = np.concatenate(shards, axis=0)  # (B, C, P)
    return full.reshape(B, C, H, W).astype(np.float32, copy=False)
